# revision 1
# baseline (speedup 1.0000x reference)
"""AttentionNCF distributed Bass kernel for 8 TRN2 NeuronCores.

Data-parallel over B=2048 (256 rows per core); rated_items and all
weights replicated.

Math note: attention scores are a rank-1 outer sum
    s[b,i] = (cand@wc)[b] + (rated@wr)[i] + att_b
and softmax over i is shift-invariant, so the cand/bias terms cancel:
    att[b,i]*um[b,i] = um[b,i] * e[i] / S[b]
with e = exp(rated@wr) and S[b] = sum_i 1[um[b,i]!=0] * e[i].
Since nonzero ratings are >= 0.5, the mask is min(2*um, 1), and
mask*e = min(2*e*um, e).

On-chip layout: activations are kept transposed ([features, batch]),
so every Linear is matmul(lhsT=W_natural, rhs=act_T) with per-partition
biases fused into the PSUM-draining relu (alternating ScalarE/VectorE).
Matmuls run in bf16 (inputs cast on-chip from the f32 DMA stream): FWL
halves the weight-load time and LDWEIGHTS pipelines with the matmul
stream, unlike fp32/fp32r whose 4-byte weight load is fused + serial.

The kernel is DMA-bound (~37MB/core): DMA emission order is the
consumption order (rated/um groups, then item weights, then
uw1/uw2/mw1..mw4), and the weight pool is deep enough that the DMA
queues never starve while towers catch up.
"""

import os

import numpy as np

import concourse.bacc as bacc
import concourse.mybir as mybir
import concourse.tile as tile
from concourse.bass import ts
from concourse.bass_utils import run_bass_kernel_spmd

F32 = mybir.dt.float32
BF16 = mybir.dt.bfloat16
AF = mybir.ActivationFunctionType
ALU = mybir.AluOpType

NCORES = 8
B, I, D = 2048, 4096, 512
BL = B // NCORES          # 256 batch rows per core
KT = I // 128             # 32 attention k-tiles
GRP = 4                   # k-tiles fetched per DMA group
NGRP = KT // GRP

# (K, M) for the dense layers
_LAYERS = {
    "iw1": (512, 1024),
    "iw2": (1024, 512),
    "uw1": (512, 2048),
    "uw2": (2048, 1024),
    "mw1": (1536, 1024),
    "mw2": (1024, 512),
    "mw3": (512, 256),
    "mw4": (256, 1),
}
# k-subtiles per DMA chunk (keeps chunks <= 1MB)
_KSUB = {"iw1": 2, "iw2": 4, "uw1": 1, "uw2": 2, "mw1": 2, "mw2": 2,
         "mw3": 4, "mw4": 2}

_CACHE = {}


def _build(niters=1):
    nc = bacc.Bacc("TRN2", target_bir_lowering=False, debug=False)

    def param(name, shape):
        return nc.declare_dram_parameter(name, list(shape), F32,
                                         isOutput=False).ap()

    candT = param("candT", (D, BL))
    umT = param("umT", (I, BL))
    rated = param("rated", (I, D))
    wrb = param("wrb", (128, D))
    w_dram = {k: param(k, (K, M)) for k, (K, M) in _LAYERS.items()}
    # all biases merged into one [128, 51] array (see _prep_host)
    biases = param("biases", (128, 51))
    out_dram = nc.declare_dram_parameter("out", [BL], F32, isOutput=True).ap()

    WBUFS = int(os.environ.get("WBUFS", "15"))

    with tile.TileContext(nc) as tc:
        with (
            tc.tile_pool(name="const", bufs=1) as cpool,
            tc.tile_pool(name="acts", bufs=1) as apool,
            tc.tile_pool(name="stream", bufs=4) as spool,
            tc.tile_pool(name="ratedbf", bufs=4) as rbfpool,
            tc.tile_pool(name="wstage", bufs=4) as wstagepool,
            tc.tile_pool(name="wtail", bufs=1) as wtailpool,
            tc.tile_pool(name="wstream", bufs=WBUFS) as wpool,
            tc.tile_pool(name="attsmall", bufs=KT + 4) as smallpool,
            tc.tile_pool(name="attwork", bufs=8) as workpool,
            tc.tile_pool(name="scratch", bufs=2) as scrpool,
            tc.tile_pool(name="psum_att", bufs=1, space="PSUM") as pa,
            tc.tile_pool(name="psum_mm", bufs=3, space="PSUM") as pm,
        ):
          for _it in range(niters):
            # ---- constants ----
            wrb_t = cpool.tile([128, D], F32, tag="wrb", name=f"wrb{_it}")
            nc.sync.dma_start(wrb_t[:], wrb[:])
            ones_t = cpool.tile([128, 128], BF16, tag="ones", name=f"ones_{_it}")
            nc.vector.memset(ones_t[:], 1.0)
            ln2_t = cpool.tile([128, 1], F32, tag="ln2", name=f"ln2_{_it}")
            nc.vector.memset(ln2_t[:], float(np.log(2.0)))
            biases_t = cpool.tile([128, 51], F32, tag="biases", name=f"biases_{_it}")
            _BOFF = {"ib1": 0, "ib2": 8, "ub1": 12, "ub2": 28, "mb1": 36,
                     "mb2": 44, "mb3": 48, "mb4": 50}

            def bias_ap(name, m):
                return biases_t[:, _BOFF[name] + m:_BOFF[name] + m + 1]

            # ---- weight streaming ----
            wtiles = {}

            def fetch_weights(name, pool=None, tag="w"):
                pool = pool or wpool
                K, M = _LAYERS[name]
                ksub = _KSUB[name]
                chunks = []
                for c in range(K // (128 * ksub)):
                    stg = wstagepool.tile([128, ksub, M], F32, tag="ws",
                                          name=f"ws_{name}_{c}_{_it}")
                    src = w_dram[name][c * 128 * ksub:(c + 1) * 128 * ksub, :]
                    nc.sync.dma_start(
                        stg[:], src.rearrange("(a p) m -> p a m", p=128))
                    t = pool.tile([128, ksub, M], BF16, tag=tag,
                                  name=f"w_{name}_{c}_{_it}")
                    # cast f32 -> bf16, split across both engines
                    if M >= 2:
                        h = M // 2
                        nc.vector.tensor_copy(t[:, :, :h], stg[:, :, :h])
                        nc.scalar.copy(t[:, :, h:], stg[:, :, h:])
                    else:
                        nc.vector.tensor_copy(t[:], stg[:])
                    chunks.append(t)
                wtiles[name] = (chunks, ksub)

            def layer_lhsT(name, kchunk, m):
                chunks, ksub = wtiles[name]
                t = chunks[kchunk // ksub]
                msz = min(128, _LAYERS[name][1])
                return t[:, kchunk % ksub, ts(m, msz)]

            # ---- dense layer: out_T[m] = relu(W.T @ x_T + b) ----
            def dense(name, x_chunks, bias_name, tag, share_tag=None):
                K, M = _LAYERS[name]
                nk, nm = K // 128, M // 128
                out_t = apool.tile([128, nm, BL], BF16,
                                   tag=share_tag or tag, name=f"act_{tag}_{_it}")
                for m in range(nm):
                    ps = pm.tile([128, BL], F32, tag="mm", name=f"ps_{tag}{m}_{_it}")
                    for k in range(nk):
                        nc.tensor.matmul(
                            ps[:], layer_lhsT(name, k, m), x_chunks[k],
                            start=(k == 0), stop=(k == nk - 1))
                    if m % 2 == 0:
                        nc.scalar.activation(
                            out_t[:, m, :], ps[:], AF.Relu,
                            bias=bias_ap(bias_name, m))
                    else:
                        nc.vector.tensor_scalar(
                            out=out_t[:, m, :], in0=ps[:],
                            scalar1=bias_ap(bias_name, m), scalar2=0.0,
                            op0=ALU.add, op1=ALU.max)
                return [out_t[:, m, :] for m in range(nm)]

            # k-outer variant: weight chunks are consumed as they arrive, so
            # after the layer's last DMA byte only nm matmuls + drains remain.
            # Uses up to 8 PSUM banks (5 from the attention pool + 3 "mm").
            def dense_kouter(name, x_chunks, bias_name, tag, share_tag=None):
                K, M = _LAYERS[name]
                nk, nm = K // 128, M // 128
                assert nm <= 8
                out_t = apool.tile([128, nm, BL], BF16,
                                   tag=share_tag or tag, name=f"act_{tag}_{_it}")
                ps_tags = [f"uf{i}" for i in range(4)] + ["s"]
                ps = []
                for m in range(nm):
                    if m < 5:
                        ps.append(pa.tile([128, BL], F32, tag=ps_tags[m],
                                          name=f"ko_{tag}{m}_{_it}"))
                    else:
                        ps.append(pm.tile([128, BL], F32, tag="mm",
                                          name=f"ko_{tag}{m}_{_it}"))
                for k in range(nk):
                    for m in range(nm):
                        nc.tensor.matmul(
                            ps[m][:], layer_lhsT(name, k, m), x_chunks[k],
                            start=(k == 0), stop=(k == nk - 1))
                for m in range(nm):
                    # alternate drains between ScalarE and VectorE so the
                    # layer-end drain doesn't serialize on one engine
                    if m % 2 == 0:
                        nc.scalar.activation(
                            out_t[:, m, :], ps[m][:], AF.Relu,
                            bias=bias_ap(bias_name, m))
                    else:
                        nc.vector.tensor_scalar(
                            out=out_t[:, m, :], in0=ps[m][:],
                            scalar1=bias_ap(bias_name, m), scalar2=0.0,
                            op0=ALU.add, op1=ALU.max)
                return [out_t[:, m, :] for m in range(nm)]

            # ---- attention phase (DMA-priority: rated/um first) ----
            uf_ps = [pa.tile([128, BL], F32, tag=f"uf{m}", name=f"uf_ps{m}_{_it}")
                     for m in range(4)]
            s_ps = pa.tile([128, BL], F32, tag="s", name=f"s_ps{_it}")

            for g in range(NGRP):
                rated_stg = spool.tile([128, GRP, D], F32, tag="rated",
                                       name=f"rated{g}_{_it}")
                nc.sync.dma_start(
                    rated_stg[:],
                    rated[g * GRP * 128:(g + 1) * GRP * 128, :]
                    .rearrange("(a p) d -> p a d", p=128))
                rated_t = rbfpool.tile([128, GRP, D], BF16, tag="ratedbf",
                                       name=f"ratedbf{g}_{_it}")
                nc.vector.tensor_copy(rated_t[:], rated_stg[:])
                um_t = spool.tile([128, GRP, BL], F32, tag="um",
                                  name=f"um{g}_{_it}")
                nc.sync.dma_start(
                    um_t[:],
                    umT[g * GRP * 128:(g + 1) * GRP * 128, :]
                    .rearrange("(a p) b -> p a b", p=128))

                for j in range(GRP):
                    k = g * GRP + j
                    # r_k[p] = sum_d rated[p,d]*wr[d]: DVE mul, ACT row-sum
                    prod = scrpool.tile([128, D], F32, tag="ttr",
                                        name=f"prod{k}_{_it}")
                    nc.vector.tensor_mul(
                        prod[:], rated_t[:, j, :], wrb_t[:])
                    prod2 = scrpool.tile([128, D], F32, tag="ttr2",
                                         name=f"prod2_{k}")
                    r_k = smallpool.tile([128, 1], F32, tag="r", name=f"r{k}_{_it}")
                    nc.scalar.activation(prod2[:], prod[:], AF.Copy,
                                         accum_out=r_k[:])
                    # e = exp(r); e2 = 2*exp(r) = exp(r + ln2)
                    e_k = smallpool.tile([128, 1], F32, tag="e", name=f"e{k}_{_it}")
                    nc.scalar.activation(e_k[:], r_k[:], AF.Exp)
                    e2_k = smallpool.tile([128, 1], F32, tag="e2",
                                          name=f"e2{k}_{_it}")
                    nc.scalar.activation(e2_k[:], r_k[:], AF.Exp,
                                         bias=ln2_t[:, 0:1])
                    # w_raw = um * e[i]
                    w_raw = workpool.tile([128, BL], BF16, tag="wraw",
                                          name=f"wraw{k}_{_it}")
                    nc.vector.tensor_scalar(
                        out=w_raw[:], in0=um_t[:, j, :],
                        scalar1=e_k[:, 0:1], scalar2=None, op0=ALU.mult)
                    # mask*e = min(2e*um, e)
                    mask_e = workpool.tile([128, BL], BF16, tag="maske",
                                           name=f"maske{k}_{_it}")
                    nc.vector.tensor_scalar(
                        out=mask_e[:], in0=um_t[:, j, :],
                        scalar1=e2_k[:, 0:1], scalar2=e_k[:, 0:1],
                        op0=ALU.mult, op1=ALU.min)
                    for m in range(4):
                        nc.tensor.matmul(
                            uf_ps[m][:],
                            rated_t[:, j, ts(m, 128)], w_raw[:],
                            start=(k == 0), stop=(k == KT - 1))
                    nc.tensor.matmul(
                        s_ps[:], ones_t[:], mask_e[:],
                        start=(k == 0), stop=(k == KT - 1))

            # ---- item tower (independent; scheduler overlaps with above)
            candT_stg = wstagepool.tile([128, 4, BL], F32, tag="ws",
                                        name=f"candT_stg{_it}")
            nc.sync.dma_start(
                candT_stg[:], candT.rearrange("(a p) b -> p a b", p=128))
            candT_t = apool.tile([128, 4, BL], BF16, tag="candT",
                                 name=f"candT_{_it}")
            nc.vector.tensor_copy(candT_t[:], candT_stg[:])
            nc.sync.dma_start(biases_t[:], biases[:])
            fetch_weights("iw1")
            fetch_weights("iw2")
            item_h1 = dense("iw1", [candT_t[:, kk, :] for kk in range(4)],
                            "ib1", tag="item_h1")
            item_emb = dense("iw2", item_h1, "ib2", tag="item_emb")

            # tail-layer weights are tiny: fetch early into own slots so the
            # kernel tail never waits on DMA
            fetch_weights("mw3", pool=wtailpool, tag="mw3")
            fetch_weights("mw4", pool=wtailpool, tag="mw4")

            # ---- S -> 1/S (guarded), uf = uf_raw / S ----
            s_sb = scrpool.tile([128, BL], F32, tag="s_sb", name=f"s_sb{_it}")
            nc.vector.tensor_scalar(
                out=s_sb[:], in0=s_ps[:], scalar1=1e-30, scalar2=None,
                op0=ALU.max)
            recip = scrpool.tile([128, BL], F32, tag="recip", name=f"recip{_it}")
            nc.vector.reciprocal(recip[:], s_sb[:])

            uf_t = apool.tile([128, 4, BL], BF16, tag="uf_sb", name=f"uf_t{_it}")  # shared w/ user_emb
            for m in range(4):
                nc.vector.tensor_tensor(
                    uf_t[:, m, :], uf_ps[m][:], recip[:], ALU.mult)
            uf_chunks = [uf_t[:, m, :] for m in range(4)]

            # ---- user tower + MLP head (weights in consumption order) ----
            fetch_weights("uw1")
            fetch_weights("uw2")
            user_h1 = dense("uw1", uf_chunks, "ub1", tag="user_h1")
            user_emb = dense_kouter("uw2", user_h1, "ub2", tag="user_emb",
                                    share_tag="uf_sb")

            fetch_weights("mw1")
            x_chunks = item_emb + user_emb
            a1 = dense_kouter("mw1", x_chunks, "mb1", tag="a1",
                              share_tag="item_h1")
            fetch_weights("mw2")
            a2 = dense_kouter("mw2", a1, "mb2", tag="a2",
                              share_tag="item_emb")
            a3 = dense("mw3", a2, "mb3", tag="a3", share_tag="candT")

            ps4 = pm.tile([128, BL], F32, tag="mm", name=f"ps4_{_it}")
            for k in range(2):
                nc.tensor.matmul(ps4[:1, :], layer_lhsT("mw4", k, 0), a3[k],
                                 start=(k == 0), stop=(k == 1))
            out_sb = scrpool.tile([1, BL], F32, tag="out_sb", name=f"out_sb{_it}")
            nc.scalar.activation(out_sb[:1, :], ps4[:1, :], AF.Identity,
                                 bias=biases_t[0:1, 50:51])
            nc.sync.dma_start(out_dram[:].rearrange("(o b) -> o b", o=1),
                              out_sb[:1, :])

    nc.compile()
    return nc


def _merge_biases(ib1, ib2, ub1, ub2, mb1, mb2, mb3, mb4):
    f = np.float32
    cols = []
    for b, n in ((ib1, 8), (ib2, 4), (ub1, 16), (ub2, 8), (mb1, 8),
                 (mb2, 4), (mb3, 2)):
        cols.append(np.asarray(b, f).reshape(n, 128).T)
    mb4col = np.zeros((128, 1), f)
    mb4col[0, 0] = np.asarray(mb4, f).reshape(())
    cols.append(mb4col)
    return np.ascontiguousarray(np.concatenate(cols, axis=1))


def _prep_host(candidate_items, rated_items, user_matrix, att_w,
               iw1, ib1, iw2, ib2, uw1, ub1, uw2, ub2,
               mw1, mb1, mw2, mb2, mw3, mb3, mw4, mb4):
    """Shard + lay out inputs for the 8 cores."""
    f = np.float32
    asc = np.ascontiguousarray

    wr = np.asarray(att_w, f)[D:, 0]                       # (512,)
    wrb = asc(np.broadcast_to(wr[None, :], (128, D)))

    shared = {
        "rated": asc(np.asarray(rated_items, f)),
        "wrb": wrb,
        "iw1": asc(np.asarray(iw1, f)), "iw2": asc(np.asarray(iw2, f)),
        "uw1": asc(np.asarray(uw1, f)), "uw2": asc(np.asarray(uw2, f)),
        "mw1": asc(np.asarray(mw1, f)), "mw2": asc(np.asarray(mw2, f)),
        "mw3": asc(np.asarray(mw3, f)), "mw4": asc(np.asarray(mw4, f)),
        "biases": _merge_biases(ib1, ib2, ub1, ub2, mb1, mb2, mb3, mb4),
    }
    cand = np.asarray(candidate_items, f)
    um = np.asarray(user_matrix, f)
    in_maps = []
    for c in range(NCORES):
        sl = slice(c * BL, (c + 1) * BL)
        m = dict(shared)
        m["candT"] = asc(cand[sl].T)
        m["umT"] = asc(um[sl].T)
        in_maps.append(m)
    return in_maps


def run(inputs, trace=False, tmpdir=None, niters=1):
    key = f"nc{niters}"
    if key not in _CACHE:
        _CACHE[key] = _build(niters)
    nc = _CACHE[key]
    in_maps = _prep_host(**{k: v for k, v in inputs.items()
                            if k not in ("att_b",)})
    res = run_bass_kernel_spmd(nc, in_maps, core_ids=list(range(NCORES)),
                               trace=trace, tmpdir=tmpdir)
    out = np.concatenate([res.results[c]["out"] for c in range(NCORES)])
    return out.reshape(B, 1).astype(np.float32), res


def kernel(**inputs):
    out, _ = run(inputs, trace=False)
    return out



# revision 6
# speedup vs baseline: 1.1304x; 1.1304x over previous
"""AttentionNCF distributed Bass kernel for 8 TRN2 NeuronCores.

Data-parallel over B=2048 (256 rows per core); rated_items and all
weights replicated. All streaming tensors are cast to bf16 on the HOST
(the kernel's matmuls already ran in bf16, so this halves DMA traffic
with no extra rounding beyond one cast).

Math note: attention scores are a rank-1 outer sum
    s[b,i] = (cand@wc)[b] + (rated@wr)[i] + att_b
and softmax over i is shift-invariant, so the cand/bias terms cancel:
    att[b,i]*um[b,i] = um[b,i] * e[i] / S[b]
with e = exp(rated@wr) and S[b] = sum_i 1[um[b,i]!=0] * e[i].
Nonzero ratings are >= 0.5, so mask*e = min(2*um*e, e) = min(2*w_raw, e).

Per k-tile of 128 rated items:
  r      = row-dot(rated, wr)     -- one fused DVE affine_mul_reduce
  e      = exp(r)                 -- ACT, batched per group
  w_raw  = um * e                 -- ACT Copy with per-partition scale
  mask_e = min(2*w_raw, e)        -- DVE tensor_scalar (4x mode, bf16)
  uf_ps[m] += rated[:,m-tile]^T @ w_raw   (4 matmuls)
  s_ps     += ones^T @ mask_e             (1 matmul)

The kernel is PE-bound (~59us of matmuls vs ~55us of DMA at the
modeled 360 B/ns), so scheduling aims to keep TensorE gap-free:
the item tower (independent of attention) is emitted in m-blocks
interleaved between attention groups as PE filler, and the user/MLP
towers run k-outer so weight chunks are consumed in arrival order.
Attention-stream DMAs ride the ACT sequencer's queue; weight DMAs ride
SP's, so a stalled weight ring never head-blocks the attention stream.
"""

import os

import numpy as np
import ml_dtypes

import concourse.bacc as bacc
import concourse.mybir as mybir
import concourse.tile as tile
from concourse.bass import ts
from concourse.bass_utils import run_bass_kernel_spmd

F32 = mybir.dt.float32
BF16 = mybir.dt.bfloat16
AF = mybir.ActivationFunctionType
ALU = mybir.AluOpType

NCORES = 8
B, I, D = 2048, 4096, 512
BL = B // NCORES          # 256 batch rows per core
KT = I // 128             # 32 attention k-tiles

# attention tile-group sizes (sum = KT); small first groups let PE start early
GROUP_SIZES = [1, 1, 2] + [4] * 7

# (K, M) for the dense layers
_LAYERS = {
    "iw1": (512, 1024),
    "iw2": (1024, 512),
    "uw1": (512, 2048),
    "uw2": (2048, 1024),
    "mw1": (1536, 1024),
    "mw2": (1024, 512),
    "mw3": (512, 256),
    "mw4": (256, 1),
}
# k-subtiles per DMA chunk (bf16 chunks, 256-512KB each)
_KSUB = {"iw1": 2, "iw2": 2, "uw1": 1, "uw2": 2, "mw1": 2, "mw2": 2,
         "mw3": 4, "mw4": 2}

# weight-chunk DMA emission schedule: group idx -> [(layer, chunk), ...]
_WSCHED = {
    0: [("iw1", 0), ("iw1", 1)],
    1: [("iw2", 0), ("iw2", 1)],
    2: [("iw2", 2), ("iw2", 3)],
    3: [("uw1", 0), ("uw1", 1)],
    4: [("uw1", 2), ("uw1", 3)],
    5: [("uw2", 0), ("uw2", 1)],
    6: [("uw2", 2), ("uw2", 3)],
    7: [("uw2", 4), ("uw2", 5)],
    8: [("uw2", 6), ("uw2", 7), ("mw1", 0)],
    9: [("mw1", 1), ("mw1", 2)],
}
_WSCHED_TAIL = [("mw1", 3), ("mw1", 4), ("mw1", 5),
                ("mw2", 0), ("mw2", 1), ("mw2", 2), ("mw2", 3)]

# item-tower m-block PE-fill schedule: group idx -> number of blocks
_FILL = {2: 2, 3: 2, 4: 2, 5: 2, 6: 1, 7: 1, 8: 0, 9: 0}
# blocks not scheduled above are emitted at the attention->user join

_CACHE = {}


def _build(niters=1):
    nc = bacc.Bacc("TRN2", target_bir_lowering=False, debug=False)

    def param(name, shape, dt=BF16):
        return nc.declare_dram_parameter(name, list(shape), dt,
                                         isOutput=False).ap()

    candT = param("candT", (D, BL))
    umT = param("umT", (I, BL))
    rated = param("rated", (I, D))
    wrb = param("wrb", (128, D))
    w_dram = {k: param(k, (K, M)) for k, (K, M) in _LAYERS.items()}
    biases = param("biases", (128, 51), F32)
    out_dram = nc.declare_dram_parameter("out", [BL], F32, isOutput=True).ap()

    WBUFS = int(os.environ.get("WBUFS", "22"))

    with tile.TileContext(nc) as tc:
        with (
            tc.tile_pool(name="const", bufs=1) as cpool,
            tc.tile_pool(name="acts", bufs=1) as apool,
            tc.tile_pool(name="stream", bufs=3) as spool,
            tc.tile_pool(name="wtail", bufs=1) as wtailpool,
            tc.tile_pool(name="wstream", bufs=WBUFS) as wpool,
            tc.tile_pool(name="attsmall", bufs=4) as smallpool,
            tc.tile_pool(name="attwork", bufs=8) as workpool,
            tc.tile_pool(name="scratch", bufs=2) as scrpool,
            tc.tile_pool(name="psum_att", bufs=1, space="PSUM") as pa,
            tc.tile_pool(name="psum_mm", bufs=3, space="PSUM") as pm,
        ):
          for _it in range(niters):
            # ---- constants ----
            wrb_t = cpool.tile([128, D], BF16, tag="wrb", name=f"wrb{_it}")
            nc.sync.dma_start(wrb_t[:], wrb[:])
            ones_t = cpool.tile([128, 128], BF16, tag="ones", name=f"ones_{_it}")
            nc.vector.memset(ones_t[:], 1.0)
            biases_t = cpool.tile([128, 51], F32, tag="biases",
                                  name=f"biases_{_it}")
            nc.sync.dma_start(biases_t[:], biases[:])
            _BOFF = {"ib1": 0, "ib2": 8, "ub1": 12, "ub2": 28, "mb1": 36,
                     "mb2": 44, "mb3": 48, "mb4": 50}

            def bias_ap(name, m):
                return biases_t[:, _BOFF[name] + m:_BOFF[name] + m + 1]

            # ---- weight streaming (bf16 direct, no staging) ----
            wtiles = {k: {} for k in _LAYERS}

            def fetch_chunk(name, c, pool=None):
                tag = "w" if pool is None else f"wt_{name}"
                pool = pool or wpool
                K, M = _LAYERS[name]
                ksub = _KSUB[name]
                t = pool.tile([128, ksub, M], BF16, tag=tag,
                              name=f"w_{name}_{c}_{_it}")
                src = w_dram[name][c * 128 * ksub:(c + 1) * 128 * ksub, :]
                nc.sync.dma_start(t[:], src.rearrange("(a p) m -> p a m", p=128))
                wtiles[name][c] = t

            def layer_lhsT(name, kchunk, m):
                ksub = _KSUB[name]
                t = wtiles[name][kchunk // ksub]
                msz = min(128, _LAYERS[name][1])
                return t[:, kchunk % ksub, ts(m, msz)]

            _drain_i = [0]

            def drain(out_ap, ps_ap, bias_name, m, engines=("act", "vec")):
                eng = engines[_drain_i[0] % len(engines)]
                _drain_i[0] += 1
                if eng == "act":
                    nc.scalar.activation(out_ap, ps_ap, AF.Relu,
                                         bias=bias_ap(bias_name, m))
                else:
                    nc.vector.tensor_scalar(
                        out=out_ap, in0=ps_ap,
                        scalar1=bias_ap(bias_name, m), scalar2=0.0,
                        op0=ALU.add, op1=ALU.max)

            # ---- m-outer dense layer (pm PSUM ring) ----
            def dense_m(name, x_chunks, bias_name, tag, share_tag=None,
                        engines=("act", "vec")):
                K, M = _LAYERS[name]
                nk, nm = K // 128, M // 128
                out_t = apool.tile([128, nm, BL], BF16,
                                   tag=share_tag or tag, name=f"act_{tag}_{_it}")
                for m in range(nm):
                    emit_dense_m_block(name, x_chunks, bias_name, out_t, m,
                                       engines)
                return [out_t[:, m, :] for m in range(nm)]

            def emit_dense_m_block(name, x_chunks, bias_name, out_t, m,
                                   engines=("act", "vec")):
                nk = _LAYERS[name][0] // 128
                ps = pm.tile([128, BL], F32, tag="mm",
                             name=f"ps_{name}{m}_{_it}")
                for k in range(nk):
                    nc.tensor.matmul(ps[:], layer_lhsT(name, k, m), x_chunks[k],
                                     start=(k == 0), stop=(k == nk - 1))
                drain(out_t[:, m, :], ps[:], bias_name, m, engines)

            # ---- k-outer dense layer (up to 8 banks: 5 pa tags + 3 pm) ----
            _KO_TAGS = ["uf0", "uf1", "uf2", "uf3", "s"]

            def dense_k(name, x_chunks, bias_name, tag, share_tag=None,
                        m_range=None, out_t=None, engines=("act", "vec")):
                K, M = _LAYERS[name]
                nk, nm = K // 128, M // 128
                if m_range is None:
                    m_range = range(nm)
                m_range = list(m_range)
                assert len(m_range) <= 8
                if out_t is None:
                    out_t = apool.tile([128, nm, BL], BF16,
                                       tag=share_tag or tag,
                                       name=f"act_{tag}_{_it}")
                ps = []
                for i, m in enumerate(m_range):
                    if i < 5:
                        ps.append(pa.tile([128, BL], F32, tag=_KO_TAGS[i],
                                          name=f"ko_{tag}{m}_{_it}"))
                    else:
                        ps.append(pm.tile([128, BL], F32, tag="mm",
                                          name=f"ko_{tag}{m}_{_it}"))
                for k in range(nk):
                    for i, m in enumerate(m_range):
                        nc.tensor.matmul(ps[i][:], layer_lhsT(name, k, m),
                                         x_chunks[k],
                                         start=(k == 0), stop=(k == nk - 1))
                for i, m in enumerate(m_range):
                    drain(out_t[:, m, :], ps[i][:], bias_name, m, engines)
                return out_t

            # ---- attention phase ----
            uf_ps = [pa.tile([128, BL], F32, tag=f"uf{m}", name=f"uf_ps{m}_{_it}")
                     for m in range(4)]
            s_ps = pa.tile([128, BL], F32, tag="s", name=f"s_ps{_it}")

            # item tower fill blocks, emitted lazily between groups
            candT_t = apool.tile([128, 4, BL], BF16, tag="candT",
                                 name=f"candT_{_it}")
            item_h1_t = apool.tile([128, 8, BL], BF16, tag="item_h1",
                                   name=f"act_item_h1_{_it}")
            item_emb_t = apool.tile([128, 4, BL], BF16, tag="item_emb",
                                    name=f"act_item_emb_{_it}")

            def mk_iw1_block(m):
                def f():
                    emit_dense_m_block(
                        "iw1", [candT_t[:, kk, :] for kk in range(4)],
                        "ib1", item_h1_t, m, engines=("act", "vec"))
                return f

            def mk_iw2_block(m):
                def f():
                    emit_dense_m_block(
                        "iw2", [item_h1_t[:, kk, :] for kk in range(8)],
                        "ib2", item_emb_t, m, engines=("act",))
                return f

            fill_blocks = [mk_iw1_block(m) for m in range(8)] + \
                          [mk_iw2_block(m) for m in range(4)]
            fill_pos = [0]

            def emit_fill(n):
                for _ in range(n):
                    if fill_pos[0] < len(fill_blocks):
                        fill_blocks[fill_pos[0]]()
                        fill_pos[0] += 1

            k0 = 0
            for g, gs in enumerate(GROUP_SIZES):
                rated_t = spool.tile([128, 4, D], BF16, tag="rated",
                                     name=f"rated{g}_{_it}")
                nc.scalar.dma_start(
                    rated_t[:, 0:gs, :],
                    rated[k0 * 128:(k0 + gs) * 128, :]
                    .rearrange("(a p) d -> p a d", p=128))
                um_t = spool.tile([128, 4, BL], BF16, tag="um",
                                  name=f"um{g}_{_it}")
                nc.scalar.dma_start(
                    um_t[:, 0:gs, :],
                    umT[k0 * 128:(k0 + gs) * 128, :]
                    .rearrange("(a p) b -> p a b", p=128))
                if g == 1:
                    nc.scalar.dma_start(
                        candT_t[:], candT.rearrange("(a p) b -> p a b", p=128))
                for (wn, wc) in _WSCHED.get(g, []):
                    fetch_chunk(wn, wc)

                # r[i] = sum_d rated[i,d]*wr[d], one fused DVE op per tile
                r_grp = smallpool.tile([128, 4], F32, tag="r", name=f"r{g}_{_it}")
                for j in range(gs):
                    junk = scrpool.tile([128, D], BF16, tag="junk",
                                        name=f"junk{g}_{j}_{_it}")
                    nc.vector.affine_mul_reduce(
                        out=junk[:], accum_out=r_grp[:, j:j + 1],
                        in0=rated_t[:, j, :], in1=wrb_t[:],
                        scale=1.0, bias=0.0)
                e_grp = smallpool.tile([128, 4], F32, tag="e", name=f"e{g}_{_it}")
                nc.scalar.activation(e_grp[:, 0:gs], r_grp[:, 0:gs], AF.Exp)

                for j in range(gs):
                    k = k0 + j
                    w_raw = workpool.tile([128, BL], BF16, tag="wraw",
                                          name=f"wraw{k}_{_it}")
                    nc.scalar.activation(w_raw[:], um_t[:, j, :], AF.Copy,
                                         scale=e_grp[:, j:j + 1])
                    mask_e = workpool.tile([128, BL], BF16, tag="maske",
                                           name=f"maske{k}_{_it}")
                    nc.vector.tensor_scalar(
                        out=mask_e[:], in0=w_raw[:],
                        scalar1=2.0, scalar2=e_grp[:, j:j + 1],
                        op0=ALU.mult, op1=ALU.min)
                    for m in range(4):
                        nc.tensor.matmul(
                            uf_ps[m][:], rated_t[:, j, ts(m, 128)], w_raw[:],
                            start=(k == 0), stop=(k == KT - 1))
                    nc.tensor.matmul(s_ps[:], ones_t[:], mask_e[:],
                                     start=(k == 0), stop=(k == KT - 1))
                emit_fill(_FILL.get(g, 0))
                k0 += gs

            # tail-layer weights + remaining weight chunks
            fetch_chunk("mw3", 0, pool=wtailpool)
            fetch_chunk("mw4", 0, pool=wtailpool)
            for (wn, wc) in _WSCHED_TAIL:
                fetch_chunk(wn, wc)

            # ---- join: uf = uf_raw / S  (remaining item blocks fill PE) ----
            emit_fill(len(fill_blocks))
            s_sb = scrpool.tile([128, BL], F32, tag="s_sb", name=f"s_sb{_it}")
            nc.vector.tensor_scalar(
                out=s_sb[:], in0=s_ps[:], scalar1=1e-30, scalar2=None,
                op0=ALU.max)
            recip = scrpool.tile([128, BL], F32, tag="recip", name=f"recip{_it}")
            nc.vector.reciprocal(recip[:], s_sb[:])
            uf_t = apool.tile([128, 4, BL], BF16, tag="uf_sb", name=f"uf_t{_it}")
            for m in range(4):
                nc.vector.tensor_tensor(
                    uf_t[:, m, :], uf_ps[m][:], recip[:], ALU.mult)
            uf_chunks = [uf_t[:, m, :] for m in range(4)]

            # ---- user tower (k-outer halves) + MLP head ----
            user_h1_t = apool.tile([128, 16, BL], BF16, tag="user_h1",
                                   name=f"act_user_h1_{_it}")
            dense_k("uw1", uf_chunks, "ub1", tag="user_h1",
                    m_range=range(0, 8), out_t=user_h1_t)
            dense_k("uw1", uf_chunks, "ub1", tag="user_h1b",
                    m_range=range(8, 16), out_t=user_h1_t)
            user_h1 = [user_h1_t[:, m, :] for m in range(16)]

            user_emb_t = dense_k("uw2", user_h1, "ub2", tag="user_emb",
                                 share_tag="uf_sb")
            user_emb = [user_emb_t[:, m, :] for m in range(8)]

            item_emb = [item_emb_t[:, m, :] for m in range(4)]
            x_chunks = item_emb + user_emb
            a1_t = dense_k("mw1", x_chunks, "mb1", tag="a1",
                           share_tag="item_h1")
            a1 = [a1_t[:, m, :] for m in range(8)]
            a2_t = dense_k("mw2", a1, "mb2", tag="a2", share_tag="item_emb")
            a2 = [a2_t[:, m, :] for m in range(4)]
            a3 = dense_m("mw3", a2, "mb3", tag="a3", share_tag="candT")

            ps4 = pm.tile([128, BL], F32, tag="mm", name=f"ps4_{_it}")
            for k in range(2):
                nc.tensor.matmul(ps4[:1, :], layer_lhsT("mw4", k, 0), a3[k],
                                 start=(k == 0), stop=(k == 1))
            out_sb = scrpool.tile([1, BL], F32, tag="out_sb", name=f"out_sb{_it}")
            nc.scalar.activation(out_sb[:1, :], ps4[:1, :], AF.Identity,
                                 bias=biases_t[0:1, 50:51])
            nc.sync.dma_start(out_dram[:].rearrange("(o b) -> o b", o=1),
                              out_sb[:1, :])

    nc.compile()
    return nc


def _merge_biases(ib1, ib2, ub1, ub2, mb1, mb2, mb3, mb4):
    f = np.float32
    cols = []
    for b, n in ((ib1, 8), (ib2, 4), (ub1, 16), (ub2, 8), (mb1, 8),
                 (mb2, 4), (mb3, 2)):
        cols.append(np.asarray(b, f).reshape(n, 128).T)
    mb4col = np.zeros((128, 1), f)
    mb4col[0, 0] = np.asarray(mb4, f).reshape(())
    cols.append(mb4col)
    return np.ascontiguousarray(np.concatenate(cols, axis=1))


def _bf(x):
    return np.ascontiguousarray(
        np.asarray(x, np.float32).astype(ml_dtypes.bfloat16))


def _prep_host(candidate_items, rated_items, user_matrix, att_w,
               iw1, ib1, iw2, ib2, uw1, ub1, uw2, ub2,
               mw1, mb1, mw2, mb2, mw3, mb3, mw4, mb4):
    """Shard + lay out inputs for the 8 cores (streams cast to bf16)."""
    f = np.float32

    wr = np.asarray(att_w, f)[D:, 0]                       # (512,)
    wrb = _bf(np.broadcast_to(wr[None, :], (128, D)))

    shared = {
        "rated": _bf(rated_items),
        "wrb": wrb,
        "iw1": _bf(iw1), "iw2": _bf(iw2),
        "uw1": _bf(uw1), "uw2": _bf(uw2),
        "mw1": _bf(mw1), "mw2": _bf(mw2),
        "mw3": _bf(mw3), "mw4": _bf(mw4),
        "biases": _merge_biases(ib1, ib2, ub1, ub2, mb1, mb2, mb3, mb4),
    }
    cand = np.asarray(candidate_items, f)
    um = np.asarray(user_matrix, f)
    in_maps = []
    for c in range(NCORES):
        sl = slice(c * BL, (c + 1) * BL)
        m = dict(shared)
        m["candT"] = _bf(cand[sl].T)
        m["umT"] = _bf(um[sl].T)
        in_maps.append(m)
    return in_maps


def run(inputs, trace=False, tmpdir=None, niters=1):
    key = f"nc{niters}"
    if key not in _CACHE:
        _CACHE[key] = _build(niters)
    nc = _CACHE[key]
    in_maps = _prep_host(**{k: v for k, v in inputs.items()
                            if k not in ("att_b",)})
    res = run_bass_kernel_spmd(nc, in_maps, core_ids=list(range(NCORES)),
                               trace=trace, tmpdir=tmpdir)
    out = np.concatenate([res.results[c]["out"] for c in range(NCORES)])
    return out.reshape(B, 1).astype(np.float32), res


def kernel(**inputs):
    out, _ = run(inputs, trace=False)
    return out


# revision 9
# speedup vs baseline: 1.4919x; 1.3199x over previous
"""AttentionNCF distributed Bass kernel for 8 TRN2 NeuronCores.

Data-parallel over B=2048 (256 rows per core); rated_items and all
weights replicated. All streaming tensors are cast to bf16 on the HOST
(the kernel's matmuls already ran in bf16, so this halves DMA traffic
with no extra rounding beyond one cast).

Math note: attention scores are a rank-1 outer sum
    s[b,i] = (cand@wc)[b] + (rated@wr)[i] + att_b
and softmax over i is shift-invariant, so the cand/bias terms cancel:
    att[b,i]*um[b,i] = um[b,i] * e[i] / S[b]
with e = exp(rated@wr) and S[b] = sum_i 1[um[b,i]!=0] * e[i].
Nonzero ratings are >= 0.5, so mask*e = min(2*um*e, e) = min(2*w_raw, e).

Per k-tile of 128 rated items:
  r      = row-dot(rated, wr)     -- one fused DVE affine_mul_reduce
  e      = exp(r)                 -- ACT, batched per group
  w_raw  = um * e                 -- ACT Copy with per-partition scale
  mask_e = min(2*w_raw, e)        -- DVE tensor_scalar (4x mode, bf16)
  uf_ps[m] += rated[:,m-tile]^T @ w_raw   (4 matmuls)
  s_ps     += ones^T @ mask_e             (1 matmul)

The kernel is PE-bound (~59us of matmuls vs ~55us of DMA at the
modeled 360 B/ns), so scheduling aims to keep TensorE gap-free:
the item tower (independent of attention) is emitted in m-blocks
interleaved between attention groups as PE filler, and the user/MLP
towers run k-outer so weight chunks are consumed in arrival order.
Attention-stream DMAs ride the ACT sequencer's queue; weight DMAs ride
SP's, so a stalled weight ring never head-blocks the attention stream.
"""

import os

import numpy as np
import ml_dtypes

import concourse.bacc as bacc
import concourse.mybir as mybir
import concourse.tile as tile
from concourse.bass import ts
from concourse.bass_utils import run_bass_kernel_spmd

F32 = mybir.dt.float32
BF16 = mybir.dt.bfloat16
AF = mybir.ActivationFunctionType
ALU = mybir.AluOpType

NCORES = 8
B, I, D = 2048, 4096, 512
BL = B // NCORES          # 256 batch rows per core
KT = I // 128             # 32 attention k-tiles

# attention tile-group sizes (sum = KT); small first groups let PE start early
GROUP_SIZES = [1, 1, 2] + [4] * 7

# (K, M) for the dense layers
_LAYERS = {
    "iw1": (512, 1024),
    "iw2": (1024, 512),
    "uw1": (512, 2048),
    "uw2": (2048, 1024),
    "mw1": (1536, 1024),
    "mw2": (1024, 512),
    "mw3": (512, 256),
    "mw4": (256, 1),
}
# k-subtiles per DMA chunk (bf16 chunks, 256-512KB each)
_KSUB = {"iw1": 2, "iw2": 4, "uw1": 1, "uw2": 4, "mw1": 4, "mw2": 4,
         "mw3": 4, "mw4": 2}

# weight-chunk DMA emission schedule: group idx -> [(layer, chunk), ...]
_WSCHED = {
    2: [("iw1", 0)],
    3: [("iw1", 1)],
    4: [("iw2", 0)],
    5: [("iw2", 1)],
}
_WSCHED_TAIL = [("uw1", 0), ("uw1", 1), ("uw1", 2), ("uw1", 3),
                ("uw2", 0), ("uw2", 1), ("uw2", 2), ("uw2", 3),
                ("mw1", 0), ("mw1", 1), ("mw1", 2),
                ("mw2", 0), ("mw2", 1)]

# item-tower m-block PE-fill schedule: group idx -> number of blocks
_FILL = {3: 2, 4: 2, 5: 2, 6: 2, 7: 1, 8: 1, 9: 0}
# blocks not scheduled above are emitted at the attention->user join

_CACHE = {}


def _build(niters=1):
    nc = bacc.Bacc("TRN2", target_bir_lowering=False, debug=False)

    def param(name, shape, dt=BF16):
        return nc.declare_dram_parameter(name, list(shape), dt,
                                         isOutput=False).ap()

    candT = param("candT", (D, BL))
    umT = param("umT", (I, BL))
    rated = param("rated", (I, D))
    wrb = param("wrb", (128, D))
    w_dram = {k: param(k, (K, M)) for k, (K, M) in _LAYERS.items()}
    biases = param("biases", (128, 51), F32)
    out_dram = nc.declare_dram_parameter("out", [BL], F32, isOutput=True).ap()

    WBUFS = int(os.environ.get("WBUFS", "17"))

    with tile.TileContext(nc) as tc:
        with (
            tc.tile_pool(name="const", bufs=1) as cpool,
            tc.tile_pool(name="acts", bufs=1) as apool,
            tc.tile_pool(name="stream", bufs=4) as spool,
            tc.tile_pool(name="wtail", bufs=1) as wtailpool,
            tc.tile_pool(name="wstream", bufs=WBUFS) as wpool,
            tc.tile_pool(name="attsmall", bufs=4) as smallpool,
            tc.tile_pool(name="attwork", bufs=8) as workpool,
            tc.tile_pool(name="scratch", bufs=2) as scrpool,
            tc.tile_pool(name="psum_att", bufs=1, space="PSUM") as pa,
            tc.tile_pool(name="psum_mm", bufs=3, space="PSUM") as pm,
        ):
          for _it in range(niters):
            # ---- constants (wrb/biases DMAs emitted inside group 0) ----
            wrb_t = cpool.tile([128, D], BF16, tag="wrb", name=f"wrb{_it}")
            ones_t = cpool.tile([128, 128], BF16, tag="ones", name=f"ones_{_it}")
            nc.vector.memset(ones_t[:], 1.0)
            biases_t = cpool.tile([128, 51], F32, tag="biases",
                                  name=f"biases_{_it}")
            _BOFF = {"ib1": 0, "ib2": 8, "ub1": 12, "ub2": 28, "mb1": 36,
                     "mb2": 44, "mb3": 48, "mb4": 50}

            def bias_ap(name, m):
                return biases_t[:, _BOFF[name] + m:_BOFF[name] + m + 1]

            # ---- weight streaming (bf16 direct, no staging) ----
            wtiles = {k: {} for k in _LAYERS}

            def fetch_chunk(name, c, pool=None):
                tag = "w" if pool is None else f"wt_{name}"
                pool = pool or wpool
                K, M = _LAYERS[name]
                ksub = _KSUB[name]
                t = pool.tile([128, ksub, M], BF16, tag=tag,
                              name=f"w_{name}_{c}_{_it}")
                src = w_dram[name][c * 128 * ksub:(c + 1) * 128 * ksub, :]
                nc.sync.dma_start(t[:], src.rearrange("(a p) m -> p a m", p=128))
                wtiles[name][c] = t

            def layer_lhsT(name, kchunk, m):
                ksub = _KSUB[name]
                t = wtiles[name][kchunk // ksub]
                msz = min(128, _LAYERS[name][1])
                return t[:, kchunk % ksub, ts(m, msz)]

            _drain_i = [0]

            def drain(out_ap, ps_ap, bias_name, m, engines=("act", "vec")):
                eng = engines[_drain_i[0] % len(engines)]
                _drain_i[0] += 1
                if eng == "act":
                    nc.scalar.activation(out_ap, ps_ap, AF.Relu,
                                         bias=bias_ap(bias_name, m))
                else:
                    nc.vector.tensor_scalar(
                        out=out_ap, in0=ps_ap,
                        scalar1=bias_ap(bias_name, m), scalar2=0.0,
                        op0=ALU.add, op1=ALU.max)

            # ---- m-outer dense layer (pm PSUM ring) ----
            def dense_m(name, x_chunks, bias_name, tag, share_tag=None,
                        engines=("act", "vec")):
                K, M = _LAYERS[name]
                nk, nm = K // 128, M // 128
                out_t = apool.tile([128, nm, BL], BF16,
                                   tag=share_tag or tag, name=f"act_{tag}_{_it}")
                for m in range(nm):
                    emit_dense_m_block(name, x_chunks, bias_name, out_t, m,
                                       engines)
                return [out_t[:, m, :] for m in range(nm)]

            def emit_dense_m_block(name, x_chunks, bias_name, out_t, m,
                                   engines=("act", "vec")):
                nk = _LAYERS[name][0] // 128
                ps = pm.tile([128, BL], F32, tag="mm",
                             name=f"ps_{name}{m}_{_it}")
                for k in range(nk):
                    nc.tensor.matmul(ps[:], layer_lhsT(name, k, m), x_chunks[k],
                                     start=(k == 0), stop=(k == nk - 1))
                drain(out_t[:, m, :], ps[:], bias_name, m, engines)

            # ---- k-outer dense layer (up to 8 banks: 5 pa tags + 3 pm) ----
            _KO_TAGS = ["uf0", "uf1", "uf2", "uf3", "s"]

            def dense_k(name, x_chunks, bias_name, tag, share_tag=None,
                        m_range=None, out_t=None, engines=("act", "vec")):
                K, M = _LAYERS[name]
                nk, nm = K // 128, M // 128
                if m_range is None:
                    m_range = range(nm)
                m_range = list(m_range)
                assert len(m_range) <= 8
                if out_t is None:
                    out_t = apool.tile([128, nm, BL], BF16,
                                       tag=share_tag or tag,
                                       name=f"act_{tag}_{_it}")
                ps = []
                for i, m in enumerate(m_range):
                    if i < 5:
                        ps.append(pa.tile([128, BL], F32, tag=_KO_TAGS[i],
                                          name=f"ko_{tag}{m}_{_it}"))
                    else:
                        ps.append(pm.tile([128, BL], F32, tag="mm",
                                          name=f"ko_{tag}{m}_{_it}"))
                for k in range(nk):
                    for i, m in enumerate(m_range):
                        nc.tensor.matmul(ps[i][:], layer_lhsT(name, k, m),
                                         x_chunks[k],
                                         start=(k == 0), stop=(k == nk - 1))
                for i, m in enumerate(m_range):
                    drain(out_t[:, m, :], ps[i][:], bias_name, m, engines)
                return out_t

            # ---- attention phase ----
            uf_ps = [pa.tile([128, BL], F32, tag=f"uf{m}", name=f"uf_ps{m}_{_it}")
                     for m in range(4)]
            s_ps = pa.tile([128, BL], F32, tag="s", name=f"s_ps{_it}")

            # item tower fill blocks, emitted lazily between groups
            candT_t = apool.tile([128, 4, BL], BF16, tag="candT",
                                 name=f"candT_{_it}")
            item_h1_t = apool.tile([128, 8, BL], BF16, tag="item_h1",
                                   name=f"act_item_h1_{_it}")
            item_emb_t = apool.tile([128, 4, BL], BF16, tag="item_emb",
                                    name=f"act_item_emb_{_it}")

            def mk_iw1_block(m):
                def f():
                    emit_dense_m_block(
                        "iw1", [candT_t[:, kk, :] for kk in range(4)],
                        "ib1", item_h1_t, m, engines=("act", "vec"))
                return f

            def mk_iw2_block(m):
                def f():
                    emit_dense_m_block(
                        "iw2", [item_h1_t[:, kk, :] for kk in range(8)],
                        "ib2", item_emb_t, m, engines=("act",))
                return f

            fill_blocks = [mk_iw1_block(m) for m in range(8)] + \
                          [mk_iw2_block(m) for m in range(4)]
            fill_pos = [0]

            def emit_fill(n):
                for _ in range(n):
                    if fill_pos[0] < len(fill_blocks):
                        fill_blocks[fill_pos[0]]()
                        fill_pos[0] += 1

            k0 = 0
            for g, gs in enumerate(GROUP_SIZES):
                rated_t = spool.tile([128, 4, D], BF16, tag="rated",
                                     name=f"rated{g}_{_it}")
                nc.sync.dma_start(
                    rated_t[:, 0:gs, :],
                    rated[k0 * 128:(k0 + gs) * 128, :]
                    .rearrange("(a p) d -> p a d", p=128))
                if g == 0:
                    nc.sync.dma_start(wrb_t[:], wrb[:])
                um_t = spool.tile([128, 4, BL], BF16, tag="um",
                                  name=f"um{g}_{_it}")
                nc.sync.dma_start(
                    um_t[:, 0:gs, :],
                    umT[k0 * 128:(k0 + gs) * 128, :]
                    .rearrange("(a p) b -> p a b", p=128))
                if g == 0:
                    nc.sync.dma_start(biases_t[:], biases[:])
                if g == 1:
                    nc.sync.dma_start(
                        candT_t[:], candT.rearrange("(a p) b -> p a b", p=128))
                for (wn, wc) in _WSCHED.get(g, []):
                    fetch_chunk(wn, wc)

                # r[i] = sum_d rated[i,d]*wr[d], one fused DVE op per tile
                r_grp = smallpool.tile([128, 4], F32, tag="r", name=f"r{g}_{_it}")
                for j in range(gs):
                    junk = scrpool.tile([128, D], BF16, tag="junk",
                                        name=f"junk{g}_{j}_{_it}")
                    nc.vector.affine_mul_reduce(
                        out=junk[:], accum_out=r_grp[:, j:j + 1],
                        in0=rated_t[:, j, :], in1=wrb_t[:],
                        scale=1.0, bias=0.0)
                e_grp = smallpool.tile([128, 4], F32, tag="e", name=f"e{g}_{_it}")
                nc.scalar.activation(e_grp[:, 0:gs], r_grp[:, 0:gs], AF.Exp)

                for j in range(gs):
                    k = k0 + j
                    w_raw = workpool.tile([128, BL], BF16, tag="wraw",
                                          name=f"wraw{k}_{_it}")
                    nc.scalar.activation(w_raw[:], um_t[:, j, :], AF.Copy,
                                         scale=e_grp[:, j:j + 1])
                    mask_e = workpool.tile([128, BL], BF16, tag="maske",
                                           name=f"maske{k}_{_it}")
                    nc.vector.tensor_scalar(
                        out=mask_e[:], in0=w_raw[:],
                        scalar1=2.0, scalar2=e_grp[:, j:j + 1],
                        op0=ALU.mult, op1=ALU.min)
                    for m in range(4):
                        nc.tensor.matmul(
                            uf_ps[m][:], rated_t[:, j, ts(m, 128)], w_raw[:],
                            start=(k == 0), stop=(k == KT - 1))
                    nc.tensor.matmul(s_ps[:], ones_t[:], mask_e[:],
                                     start=(k == 0), stop=(k == KT - 1))
                emit_fill(_FILL.get(g, 0))
                k0 += gs

            # tail-layer weights + remaining weight chunks
            fetch_chunk("mw3", 0, pool=wtailpool)
            fetch_chunk("mw4", 0, pool=wtailpool)
            for (wn, wc) in _WSCHED_TAIL:
                fetch_chunk(wn, wc)

            # ---- join: uf = uf_raw / S  (remaining item blocks fill PE) ----
            emit_fill(len(fill_blocks))
            s_sb = scrpool.tile([128, BL], F32, tag="s_sb", name=f"s_sb{_it}")
            nc.vector.tensor_scalar(
                out=s_sb[:], in0=s_ps[:], scalar1=1e-30, scalar2=None,
                op0=ALU.max)
            recip = scrpool.tile([128, BL], F32, tag="recip", name=f"recip{_it}")
            nc.vector.reciprocal(recip[:], s_sb[:])
            uf_t = apool.tile([128, 4, BL], BF16, tag="uf_sb", name=f"uf_t{_it}")
            for m in range(4):
                nc.vector.tensor_tensor(
                    uf_t[:, m, :], uf_ps[m][:], recip[:], ALU.mult)
            uf_chunks = [uf_t[:, m, :] for m in range(4)]

            # ---- user tower (k-outer halves) + MLP head ----
            user_h1_t = apool.tile([128, 16, BL], BF16, tag="user_h1",
                                   name=f"act_user_h1_{_it}")
            dense_k("uw1", uf_chunks, "ub1", tag="user_h1",
                    m_range=range(0, 8), out_t=user_h1_t)
            dense_k("uw1", uf_chunks, "ub1", tag="user_h1b",
                    m_range=range(8, 16), out_t=user_h1_t)
            user_h1 = [user_h1_t[:, m, :] for m in range(16)]

            user_emb_t = dense_k("uw2", user_h1, "ub2", tag="user_emb",
                                 share_tag="uf_sb")
            user_emb = [user_emb_t[:, m, :] for m in range(8)]

            item_emb = [item_emb_t[:, m, :] for m in range(4)]
            x_chunks = item_emb + user_emb
            a1_t = dense_k("mw1", x_chunks, "mb1", tag="a1",
                           share_tag="item_h1")
            a1 = [a1_t[:, m, :] for m in range(8)]
            a2_t = dense_k("mw2", a1, "mb2", tag="a2", share_tag="item_emb")
            a2 = [a2_t[:, m, :] for m in range(4)]
            a3 = dense_m("mw3", a2, "mb3", tag="a3", share_tag="candT")

            ps4 = pm.tile([128, BL], F32, tag="mm", name=f"ps4_{_it}")
            for k in range(2):
                nc.tensor.matmul(ps4[:1, :], layer_lhsT("mw4", k, 0), a3[k],
                                 start=(k == 0), stop=(k == 1))
            out_sb = scrpool.tile([1, BL], F32, tag="out_sb", name=f"out_sb{_it}")
            nc.scalar.activation(out_sb[:1, :], ps4[:1, :], AF.Identity,
                                 bias=biases_t[0:1, 50:51])
            nc.sync.dma_start(out_dram[:].rearrange("(o b) -> o b", o=1),
                              out_sb[:1, :])

    nc.compile()
    return nc


def _merge_biases(ib1, ib2, ub1, ub2, mb1, mb2, mb3, mb4):
    f = np.float32
    cols = []
    for b, n in ((ib1, 8), (ib2, 4), (ub1, 16), (ub2, 8), (mb1, 8),
                 (mb2, 4), (mb3, 2)):
        cols.append(np.asarray(b, f).reshape(n, 128).T)
    mb4col = np.zeros((128, 1), f)
    mb4col[0, 0] = np.asarray(mb4, f).reshape(())
    cols.append(mb4col)
    return np.ascontiguousarray(np.concatenate(cols, axis=1))


def _bf(x):
    return np.ascontiguousarray(
        np.asarray(x, np.float32).astype(ml_dtypes.bfloat16))


def _prep_host(candidate_items, rated_items, user_matrix, att_w,
               iw1, ib1, iw2, ib2, uw1, ub1, uw2, ub2,
               mw1, mb1, mw2, mb2, mw3, mb3, mw4, mb4):
    """Shard + lay out inputs for the 8 cores (streams cast to bf16)."""
    f = np.float32

    wr = np.asarray(att_w, f)[D:, 0]                       # (512,)
    wrb = _bf(np.broadcast_to(wr[None, :], (128, D)))

    shared = {
        "rated": _bf(rated_items),
        "wrb": wrb,
        "iw1": _bf(iw1), "iw2": _bf(iw2),
        "uw1": _bf(uw1), "uw2": _bf(uw2),
        "mw1": _bf(mw1), "mw2": _bf(mw2),
        "mw3": _bf(mw3), "mw4": _bf(mw4),
        "biases": _merge_biases(ib1, ib2, ub1, ub2, mb1, mb2, mb3, mb4),
    }
    cand = np.asarray(candidate_items, f)
    um = np.asarray(user_matrix, f)
    in_maps = []
    for c in range(NCORES):
        sl = slice(c * BL, (c + 1) * BL)
        m = dict(shared)
        m["candT"] = _bf(cand[sl].T)
        m["umT"] = _bf(um[sl].T)
        in_maps.append(m)
    return in_maps


def run(inputs, trace=False, tmpdir=None, niters=1):
    key = f"nc{niters}"
    if key not in _CACHE:
        _CACHE[key] = _build(niters)
    nc = _CACHE[key]
    in_maps = _prep_host(**{k: v for k, v in inputs.items()
                            if k not in ("att_b",)})
    res = run_bass_kernel_spmd(nc, in_maps, core_ids=list(range(NCORES)),
                               trace=trace, tmpdir=tmpdir)
    out = np.concatenate([res.results[c]["out"] for c in range(NCORES)])
    return out.reshape(B, 1).astype(np.float32), res


def kernel(**inputs):
    out, _ = run(inputs, trace=False)
    return out


# revision 11
# speedup vs baseline: 1.5415x; 1.0332x over previous
"""AttentionNCF distributed Bass kernel for 8 TRN2 NeuronCores.

Data-parallel over B=2048 (256 rows per core); rated_items and all
weights replicated. All streaming tensors are cast to bf16 on the HOST
(the kernel's matmuls already ran in bf16, so this halves DMA traffic
with no extra rounding beyond one cast).

Math note: attention scores are a rank-1 outer sum
    s[b,i] = (cand@wc)[b] + (rated@wr)[i] + att_b
and softmax over i is shift-invariant, so the cand/bias terms cancel:
    att[b,i]*um[b,i] = um[b,i] * e[i] / S[b]
with e = exp(rated@wr) and S[b] = sum_i 1[um[b,i]!=0] * e[i].
Nonzero ratings are >= 0.5, so mask*e = min(2*um*e, e) = min(2*w_raw, e).

Per k-tile of 128 rated items:
  r      = row-dot(rated, wr)     -- one fused DVE affine_mul_reduce
  e      = exp(r)                 -- ACT, batched per group
  w_raw  = um * e                 -- ACT Copy with per-partition scale
  mask_e = min(2*w_raw, e)        -- DVE tensor_scalar (4x mode, bf16)
  uf_ps[m] += rated[:,m-tile]^T @ w_raw   (4 matmuls)
  s_ps     += ones^T @ mask_e             (1 matmul)

The kernel is PE-bound (~59us of matmuls vs ~55us of DMA at the
modeled 360 B/ns), so scheduling aims to keep TensorE gap-free:
the item tower (independent of attention) is emitted in m-blocks
interleaved between attention groups as PE filler, and the user/MLP
towers run k-outer so weight chunks are consumed in arrival order.
Attention-stream DMAs ride the ACT sequencer's queue; weight DMAs ride
SP's, so a stalled weight ring never head-blocks the attention stream.
"""

import os

import numpy as np
import ml_dtypes

import concourse.bacc as bacc
import concourse.mybir as mybir
import concourse.tile as tile
from concourse.bass import ts
from concourse.bass_utils import run_bass_kernel_spmd

F32 = mybir.dt.float32
BF16 = mybir.dt.bfloat16
AF = mybir.ActivationFunctionType
ALU = mybir.AluOpType

NCORES = 8
B, I, D = 2048, 4096, 512
BL = B // NCORES          # 256 batch rows per core
KT = I // 128             # 32 attention k-tiles

# attention tile-group sizes (sum = KT); small first groups let PE start early
GROUP_SIZES = [1, 1, 2] + [4] * 7

# (K, M) for the dense layers
_LAYERS = {
    "iw1": (512, 1024),
    "iw2": (1024, 512),
    "uw1": (512, 2048),
    "uw2": (2048, 1024),
    "mw1": (1536, 1024),
    "mw2": (1024, 512),
    "mw3": (512, 256),
    "mw4": (256, 1),
}
# k-subtiles per DMA chunk (bf16 chunks, 256-512KB each)
_KSUB = {"iw1": 2, "iw2": 4, "uw1": 1, "uw2": 4, "mw1": 4, "mw2": 4,
         "mw3": 4, "mw4": 2}

# weight-chunk DMA emission schedule: group idx -> [(layer, chunk), ...]
_WSCHED = {
    1: [("iw1", 0)],
    2: [("iw1", 1)],
    3: [("iw2", 0)],
    4: [("iw2", 1)],
}
_WSCHED_TAIL = [("uw1", 0), ("uw1", 1), ("uw1", 2), ("uw1", 3),
                ("uw2", 0), ("uw2", 1), ("uw2", 2), ("uw2", 3),
                ("mw1", 0), ("mw1", 1), ("mw1", 2),
                ("mw2", 0), ("mw2", 1)]

# item-tower m-block PE-fill schedule: group idx -> number of blocks
_FILL = {2: 2, 3: 1, 4: 2, 5: 1, 6: 2, 7: 1, 8: 1, 9: 0}
# blocks not scheduled above are emitted at the attention->user join

_CACHE = {}


def _build(niters=1):
    nc = bacc.Bacc("TRN2", target_bir_lowering=False, debug=False)

    def param(name, shape, dt=BF16):
        return nc.declare_dram_parameter(name, list(shape), dt,
                                         isOutput=False).ap()

    candT = param("candT", (D, BL))
    umT = param("umT", (I, BL))
    rated = param("rated", (I, D))
    wrb = param("wrb", (128, D))
    w_dram = {k: param(k, (K, M)) for k, (K, M) in _LAYERS.items()}
    biases = param("biases", (128, 51), F32)
    out_dram = nc.declare_dram_parameter("out", [BL], F32, isOutput=True).ap()

    WBUFS = int(os.environ.get("WBUFS", "17"))

    with tile.TileContext(nc) as tc:
        with (
            tc.tile_pool(name="const", bufs=1) as cpool,
            tc.tile_pool(name="acts", bufs=1) as apool,
            tc.tile_pool(name="stream", bufs=4) as spool,
            tc.tile_pool(name="wtail", bufs=1) as wtailpool,
            tc.tile_pool(name="wstream", bufs=WBUFS) as wpool,
            tc.tile_pool(name="attsmall", bufs=4) as smallpool,
            tc.tile_pool(name="attwork", bufs=8) as workpool,
            tc.tile_pool(name="scratch", bufs=2) as scrpool,
            tc.tile_pool(name="psum_att", bufs=1, space="PSUM") as pa,
            tc.tile_pool(name="psum_mm", bufs=3, space="PSUM") as pm,
        ):
          for _it in range(niters):
            # ---- constants (wrb/biases DMAs emitted inside group 0) ----
            wrb_t = cpool.tile([128, D], BF16, tag="wrb", name=f"wrb{_it}")
            ones_t = cpool.tile([128, 128], BF16, tag="ones", name=f"ones_{_it}")
            nc.vector.memset(ones_t[:], 1.0)
            biases_t = cpool.tile([128, 51], F32, tag="biases",
                                  name=f"biases_{_it}")
            _BOFF = {"ib1": 0, "ib2": 8, "ub1": 12, "ub2": 28, "mb1": 36,
                     "mb2": 44, "mb3": 48, "mb4": 50}

            def bias_ap(name, m):
                return biases_t[:, _BOFF[name] + m:_BOFF[name] + m + 1]

            # ---- weight streaming (bf16 direct, no staging) ----
            wtiles = {k: {} for k in _LAYERS}

            def fetch_chunk(name, c, pool=None):
                tag = "w" if pool is None else f"wt_{name}"
                pool = pool or wpool
                K, M = _LAYERS[name]
                ksub = _KSUB[name]
                t = pool.tile([128, ksub, M], BF16, tag=tag,
                              name=f"w_{name}_{c}_{_it}")
                src = w_dram[name][c * 128 * ksub:(c + 1) * 128 * ksub, :]
                nc.sync.dma_start(t[:], src.rearrange("(a p) m -> p a m", p=128))
                wtiles[name][c] = t

            def layer_lhsT(name, kchunk, m):
                ksub = _KSUB[name]
                t = wtiles[name][kchunk // ksub]
                msz = min(128, _LAYERS[name][1])
                return t[:, kchunk % ksub, ts(m, msz)]

            _drain_i = [0]

            def drain(out_ap, ps_ap, bias_name, m, engines=("act", "vec")):
                eng = engines[_drain_i[0] % len(engines)]
                _drain_i[0] += 1
                if eng == "act":
                    nc.scalar.activation(out_ap, ps_ap, AF.Relu,
                                         bias=bias_ap(bias_name, m))
                else:
                    nc.vector.tensor_scalar(
                        out=out_ap, in0=ps_ap,
                        scalar1=bias_ap(bias_name, m), scalar2=0.0,
                        op0=ALU.add, op1=ALU.max)

            # ---- m-outer dense layer (pm PSUM ring) ----
            def dense_m(name, x_chunks, bias_name, tag, share_tag=None,
                        engines=("act", "vec")):
                K, M = _LAYERS[name]
                nk, nm = K // 128, M // 128
                out_t = apool.tile([128, nm, BL], BF16,
                                   tag=share_tag or tag, name=f"act_{tag}_{_it}")
                for m in range(nm):
                    emit_dense_m_block(name, x_chunks, bias_name, out_t, m,
                                       engines)
                return [out_t[:, m, :] for m in range(nm)]

            def emit_dense_m_block(name, x_chunks, bias_name, out_t, m,
                                   engines=("act", "vec")):
                nk = _LAYERS[name][0] // 128
                ps = pm.tile([128, BL], F32, tag="mm",
                             name=f"ps_{name}{m}_{_it}")
                for k in range(nk):
                    nc.tensor.matmul(ps[:], layer_lhsT(name, k, m), x_chunks[k],
                                     start=(k == 0), stop=(k == nk - 1))
                drain(out_t[:, m, :], ps[:], bias_name, m, engines)

            # ---- k-outer dense layer (up to 8 banks: 5 pa tags + 3 pm) ----
            _KO_TAGS = ["uf0", "uf1", "uf2", "uf3", "s"]

            def dense_k(name, x_chunks, bias_name, tag, share_tag=None,
                        m_range=None, out_t=None, engines=("act", "vec")):
                K, M = _LAYERS[name]
                nk, nm = K // 128, M // 128
                if m_range is None:
                    m_range = range(nm)
                m_range = list(m_range)
                assert len(m_range) <= 8
                if out_t is None:
                    out_t = apool.tile([128, nm, BL], BF16,
                                       tag=share_tag or tag,
                                       name=f"act_{tag}_{_it}")
                ps = []
                for i, m in enumerate(m_range):
                    if i < 5:
                        ps.append(pa.tile([128, BL], F32, tag=_KO_TAGS[i],
                                          name=f"ko_{tag}{m}_{_it}"))
                    else:
                        ps.append(pm.tile([128, BL], F32, tag="mm",
                                          name=f"ko_{tag}{m}_{_it}"))
                for k in range(nk):
                    for i, m in enumerate(m_range):
                        nc.tensor.matmul(ps[i][:], layer_lhsT(name, k, m),
                                         x_chunks[k],
                                         start=(k == 0), stop=(k == nk - 1))
                for i, m in enumerate(m_range):
                    drain(out_t[:, m, :], ps[i][:], bias_name, m, engines)
                return out_t

            # ---- attention phase ----
            uf_ps = [pa.tile([128, BL], F32, tag=f"uf{m}", name=f"uf_ps{m}_{_it}")
                     for m in range(4)]
            s_ps = pa.tile([128, BL], F32, tag="s", name=f"s_ps{_it}")

            # item tower fill blocks, emitted lazily between groups
            candT_t = apool.tile([128, 4, BL], BF16, tag="candT",
                                 name=f"candT_{_it}")
            item_h1_t = apool.tile([128, 8, BL], BF16, tag="item_h1",
                                   name=f"act_item_h1_{_it}")
            item_emb_t = apool.tile([128, 4, BL], BF16, tag="item_emb",
                                    name=f"act_item_emb_{_it}")

            def mk_iw1_block(m):
                def f():
                    emit_dense_m_block(
                        "iw1", [candT_t[:, kk, :] for kk in range(4)],
                        "ib1", item_h1_t, m, engines=("act", "vec"))
                return f

            def mk_iw2_block(m):
                def f():
                    emit_dense_m_block(
                        "iw2", [item_h1_t[:, kk, :] for kk in range(8)],
                        "ib2", item_emb_t, m, engines=("act",))
                return f

            fill_blocks = [mk_iw1_block(m) for m in range(8)] + \
                          [mk_iw2_block(m) for m in range(4)]
            fill_pos = [0]

            def emit_fill(n):
                for _ in range(n):
                    if fill_pos[0] < len(fill_blocks):
                        fill_blocks[fill_pos[0]]()
                        fill_pos[0] += 1

            k0 = 0
            for g, gs in enumerate(GROUP_SIZES):
                rated_t = spool.tile([128, 4, D], BF16, tag="rated",
                                     name=f"rated{g}_{_it}")
                nc.sync.dma_start(
                    rated_t[:, 0:gs, :],
                    rated[k0 * 128:(k0 + gs) * 128, :]
                    .rearrange("(a p) d -> p a d", p=128))
                if g == 0:
                    nc.sync.dma_start(wrb_t[:], wrb[:])
                um_t = spool.tile([128, 4, BL], BF16, tag="um",
                                  name=f"um{g}_{_it}")
                nc.sync.dma_start(
                    um_t[:, 0:gs, :],
                    umT[k0 * 128:(k0 + gs) * 128, :]
                    .rearrange("(a p) b -> p a b", p=128))
                if g == 1:
                    nc.sync.dma_start(biases_t[:], biases[:])
                if g == 2:
                    nc.sync.dma_start(
                        candT_t[:], candT.rearrange("(a p) b -> p a b", p=128))
                for (wn, wc) in _WSCHED.get(g, []):
                    fetch_chunk(wn, wc)

                # r[i] = sum_d rated[i,d]*wr[d], one fused DVE op per tile
                r_grp = smallpool.tile([128, 4], F32, tag="r", name=f"r{g}_{_it}")
                for j in range(gs):
                    junk = scrpool.tile([128, D], BF16, tag="junk",
                                        name=f"junk{g}_{j}_{_it}")
                    nc.vector.affine_mul_reduce(
                        out=junk[:], accum_out=r_grp[:, j:j + 1],
                        in0=rated_t[:, j, :], in1=wrb_t[:],
                        scale=1.0, bias=0.0)
                e_grp = smallpool.tile([128, 4], F32, tag="e", name=f"e{g}_{_it}")
                nc.scalar.activation(e_grp[:, 0:gs], r_grp[:, 0:gs], AF.Exp)

                mask_tiles = []
                for j in range(gs):
                    k = k0 + j
                    w_raw = workpool.tile([128, BL], BF16, tag="wraw",
                                          name=f"wraw{k}_{_it}")
                    nc.scalar.activation(w_raw[:], um_t[:, j, :], AF.Copy,
                                         scale=e_grp[:, j:j + 1])
                    mask_e = workpool.tile([128, BL], BF16, tag="maske",
                                           name=f"maske{k}_{_it}")
                    nc.gpsimd.tensor_scalar(
                        out=mask_e[:], in0=w_raw[:],
                        scalar1=2.0, scalar2=e_grp[:, j:j + 1],
                        op0=ALU.mult, op1=ALU.min)
                    mask_tiles.append(mask_e)
                    for m in range(4):
                        nc.tensor.matmul(
                            uf_ps[m][:], rated_t[:, j, ts(m, 128)], w_raw[:],
                            start=(k == 0), stop=(k == KT - 1))
                for j in range(gs):
                    k = k0 + j
                    nc.tensor.matmul(s_ps[:], ones_t[:], mask_tiles[j][:],
                                     start=(k == 0), stop=(k == KT - 1))
                emit_fill(_FILL.get(g, 0))
                k0 += gs

            # tail-layer weights + remaining weight chunks
            fetch_chunk("mw3", 0, pool=wtailpool)
            fetch_chunk("mw4", 0, pool=wtailpool)
            for (wn, wc) in _WSCHED_TAIL:
                fetch_chunk(wn, wc)

            # ---- join: uf = uf_raw / S  (remaining item blocks fill PE) ----
            emit_fill(len(fill_blocks))
            s_sb = scrpool.tile([128, BL], F32, tag="s_sb", name=f"s_sb{_it}")
            nc.vector.tensor_scalar(
                out=s_sb[:], in0=s_ps[:], scalar1=1e-30, scalar2=None,
                op0=ALU.max)
            recip = scrpool.tile([128, BL], F32, tag="recip", name=f"recip{_it}")
            nc.vector.reciprocal(recip[:], s_sb[:])
            uf_t = apool.tile([128, 4, BL], BF16, tag="uf_sb", name=f"uf_t{_it}")
            for m in range(4):
                nc.vector.tensor_tensor(
                    uf_t[:, m, :], uf_ps[m][:], recip[:], ALU.mult)
            uf_chunks = [uf_t[:, m, :] for m in range(4)]

            # ---- user tower (k-outer halves) + MLP head ----
            user_h1_t = apool.tile([128, 16, BL], BF16, tag="user_h1",
                                   name=f"act_user_h1_{_it}")
            dense_k("uw1", uf_chunks, "ub1", tag="user_h1",
                    m_range=range(0, 8), out_t=user_h1_t)
            dense_k("uw1", uf_chunks, "ub1", tag="user_h1b",
                    m_range=range(8, 16), out_t=user_h1_t)
            user_h1 = [user_h1_t[:, m, :] for m in range(16)]

            user_emb_t = dense_k("uw2", user_h1, "ub2", tag="user_emb",
                                 share_tag="uf_sb")
            user_emb = [user_emb_t[:, m, :] for m in range(8)]

            item_emb = [item_emb_t[:, m, :] for m in range(4)]
            x_chunks = item_emb + user_emb
            a1_t = dense_k("mw1", x_chunks, "mb1", tag="a1",
                           share_tag="item_h1")
            a1 = [a1_t[:, m, :] for m in range(8)]
            a2_t = dense_k("mw2", a1, "mb2", tag="a2", share_tag="item_emb")
            a2 = [a2_t[:, m, :] for m in range(4)]
            a3 = dense_m("mw3", a2, "mb3", tag="a3", share_tag="candT")

            ps4 = pm.tile([128, BL], F32, tag="mm", name=f"ps4_{_it}")
            for k in range(2):
                nc.tensor.matmul(ps4[:1, :], layer_lhsT("mw4", k, 0), a3[k],
                                 start=(k == 0), stop=(k == 1))
            out_sb = scrpool.tile([1, BL], F32, tag="out_sb", name=f"out_sb{_it}")
            nc.scalar.activation(out_sb[:1, :], ps4[:1, :], AF.Identity,
                                 bias=biases_t[0:1, 50:51])
            nc.sync.dma_start(out_dram[:].rearrange("(o b) -> o b", o=1),
                              out_sb[:1, :])

    nc.compile()
    return nc


def _merge_biases(ib1, ib2, ub1, ub2, mb1, mb2, mb3, mb4):
    f = np.float32
    cols = []
    for b, n in ((ib1, 8), (ib2, 4), (ub1, 16), (ub2, 8), (mb1, 8),
                 (mb2, 4), (mb3, 2)):
        cols.append(np.asarray(b, f).reshape(n, 128).T)
    mb4col = np.zeros((128, 1), f)
    mb4col[0, 0] = np.asarray(mb4, f).reshape(())
    cols.append(mb4col)
    return np.ascontiguousarray(np.concatenate(cols, axis=1))


def _bf(x):
    return np.ascontiguousarray(
        np.asarray(x, np.float32).astype(ml_dtypes.bfloat16))


def _prep_host(candidate_items, rated_items, user_matrix, att_w,
               iw1, ib1, iw2, ib2, uw1, ub1, uw2, ub2,
               mw1, mb1, mw2, mb2, mw3, mb3, mw4, mb4):
    """Shard + lay out inputs for the 8 cores (streams cast to bf16)."""
    f = np.float32

    wr = np.asarray(att_w, f)[D:, 0]                       # (512,)
    wrb = _bf(np.broadcast_to(wr[None, :], (128, D)))

    shared = {
        "rated": _bf(rated_items),
        "wrb": wrb,
        "iw1": _bf(iw1), "iw2": _bf(iw2),
        "uw1": _bf(uw1), "uw2": _bf(uw2),
        "mw1": _bf(mw1), "mw2": _bf(mw2),
        "mw3": _bf(mw3), "mw4": _bf(mw4),
        "biases": _merge_biases(ib1, ib2, ub1, ub2, mb1, mb2, mb3, mb4),
    }
    cand = np.asarray(candidate_items, f)
    um = np.asarray(user_matrix, f)
    in_maps = []
    for c in range(NCORES):
        sl = slice(c * BL, (c + 1) * BL)
        m = dict(shared)
        m["candT"] = _bf(cand[sl].T)
        m["umT"] = _bf(um[sl].T)
        in_maps.append(m)
    return in_maps


def run(inputs, trace=False, tmpdir=None, niters=1):
    key = f"nc{niters}"
    if key not in _CACHE:
        _CACHE[key] = _build(niters)
    nc = _CACHE[key]
    in_maps = _prep_host(**{k: v for k, v in inputs.items()
                            if k not in ("att_b",)})
    res = run_bass_kernel_spmd(nc, in_maps, core_ids=list(range(NCORES)),
                               trace=trace, tmpdir=tmpdir)
    out = np.concatenate([res.results[c]["out"] for c in range(NCORES)])
    return out.reshape(B, 1).astype(np.float32), res


def kernel(**inputs):
    out, _ = run(inputs, trace=False)
    return out
